# revision 18
# baseline (speedup 1.0000x reference)
"""DeformConv2d (DCNv2) Trainium2 Bass kernel, v2.

Problem: N=4, C_IN=C_OUT=64, H=W=128, 3x3 taps, stride=1, pad=1, dil=1,
modulated deformable conv (torchvision semantics).

Sharding: 8 cores; core = (image n = core//2, row-half = core%2).
Each core computes out[n, :, i0:i0+64, :] from the full image x[n].

v2 design (vs v1 baseline):
  - bf16 "quad" image in DRAM: entry (y,x) holds the 2x2 pixel block
    (y..y+1, x..x+1) x 64ch as 256 bf16 values ordered (c, q) with
    q = yc*2+xc.  One 512B gather descriptor fetches all 4 bilinear
    corners of one (pixel, tap) sample -> half the descriptors and half
    the HBM bytes of v1.
  - index repack j-major -> 16-partition-wrapped via two PE transpose
    stages (v1 used a DRAM bounce with 256B descriptors: ~460us).
  - corner combine: one 2x-mode DVE multiply (weights broadcast over
    channels via stride-0 AP) + one tensor_reduce over the 4-corner
    inner axis.
  - conv: row-pair transposes ([128j, 128(i2,c)] -> [(i2,c), j]) and
    per-tap 128-contraction matmuls with block-diagonal duplicated
    weights -> 2 output pixels per PE column.
"""
import sys

_TRN_REPO = "/opt/trn_rl_repo"
if _TRN_REPO not in sys.path:
    sys.path.insert(0, _TRN_REPO)

import numpy as np
import ml_dtypes

import concourse.bass as bass
import concourse.bacc as bacc
import concourse.tile as tile
import concourse.mybir as mybir
from concourse.bass_utils import run_bass_kernel_spmd
from contextlib import ExitStack

F32 = mybir.dt.float32
BF16 = mybir.dt.bfloat16
I16 = mybir.dt.int16
ALU = mybir.AluOpType
BF = ml_dtypes.bfloat16

N, C, H, W = 4, 64, 128, 128
K2 = 9
PAD = 16                    # coordinate padding on each side
PH = H + 2 * PAD            # 160
PW = W + 2 * PAD            # 160
NQ = PH * PW                # 25600 quad entries
HI = 64                     # rows per core
R = 16                      # rows per gather block
NBLK = HI // R              # 4
RSUB = 8                    # rows per dma_gather call (HW caps 1024 descs)
NIDX = RSUB * W             # descriptors per dma_gather call
CLAMP = 11.0                # |floor(offset)| clamp (pad-region safe)
MAGIC = 12582912.0          # 1.5 * 2**23 for round-to-nearest-even
DMA_SCRATCH = 49152         # SWDGE descriptor carveout (ring = this/16)

_CACHED = {}


def build_nc():
    nc = bacc.Bacc(trn_type="TRN2", debug=False, num_swdge_queues=4,
                   dynamic_dma_scratch_size=DMA_SCRATCH)

    xq_d = nc.dram_tensor("xq", [NQ * 256], BF16, kind="ExternalInput")
    offj_d = nc.dram_tensor("offj", [128, 2 * K2 * HI], F32, kind="ExternalInput").ap()
    maskj_d = nc.dram_tensor("maskj", [128, K2 * HI], F32, kind="ExternalInput").ap()
    idxb_d = nc.dram_tensor("idxb", [16 * K2 * HI * 8], F32, kind="ExternalInput")
    wk2_d = nc.dram_tensor("wk2", [128, K2 * 128], BF16, kind="ExternalInput").ap()
    identf_d = nc.dram_tensor("identf", [128, 128], F32, kind="ExternalInput").ap()
    identb_d = nc.dram_tensor("identb", [128, 128], BF16, kind="ExternalInput").ap()
    out_d = nc.dram_tensor("out", [64, HI * W], F32, kind="ExternalOutput").ap()

    # gather source: quad entries of the padded image
    src_ap = bass.AP(xq_d, 0, [[256, NQ], [1, 256]])

    NM = K2 * HI            # 576 (k, i) pairs
    NFREE = NM * 8          # 4608 descriptors per 16-partition wrap

    with ExitStack() as ctx:
        tc = ctx.enter_context(tile.TileContext(nc))

        const = ctx.enter_context(tc.tile_pool(name="const", bufs=1))
        live = ctx.enter_context(tc.tile_pool(name="live", bufs=1))
        ph1 = ExitStack()
        work = ph1.enter_context(tc.tile_pool(name="work", bufs=1))
        ps1pool = ph1.enter_context(tc.tile_pool(name="ps1", bufs=2, space="PSUM"))
        ps2pool = ph1.enter_context(tc.tile_pool(name="ps2", bufs=2, space="PSUM"))

        identf = const.tile([128, 128], F32)
        nc.sync.dma_start(identf[:], identf_d)
        identb = const.tile([128, 128], BF16)
        nc.sync.dma_start(identb[:], identb_d)
        wk2 = const.tile([128, K2 * 128], BF16)
        nc.sync.dma_start(wk2[:], wk2_d)

        offj = work.tile([128, 2 * K2 * HI], F32)
        nc.sync.dma_start(offj[:], offj_d)
        maskj = work.tile([128, K2 * HI], F32)
        nc.sync.dma_start(maskj[:], maskj_d)
        idxb = work.tile([128, NFREE], F32)
        nc.sync.dma_start(idxb[0:16, :], bass.AP(idxb_d, 0, [[NFREE, 16], [1, NFREE]]))

        # ---- Phase 1a: floor / frac ------------------------------------
        flo = work.tile([128, 2 * K2 * HI], F32)
        nc.vector.tensor_scalar(flo[:], offj[:], MAGIC, None, ALU.add)
        nc.vector.tensor_scalar(flo[:], flo[:], MAGIC, None, ALU.subtract)
        rup = work.tile([128, 2 * K2 * HI], F32)
        nc.vector.tensor_tensor(rup[:], flo[:], offj[:], ALU.is_gt)
        nc.vector.tensor_tensor(flo[:], flo[:], rup[:], ALU.subtract)
        frac = work.tile([128, 2 * K2 * HI], F32)
        nc.vector.tensor_tensor(frac[:], offj[:], flo[:], ALU.subtract)
        nc.vector.tensor_scalar(flo[:], flo[:], -CLAMP, None, ALU.max)
        nc.vector.tensor_scalar(flo[:], flo[:], CLAMP, None, ALU.min)

        def kv(t):  # [128, (k, two, i)]
            return t[:].rearrange("p (k two i) -> p k two i", k=K2, two=2, i=HI)

        # dyx[j, m=(k,i)] = floor(dy)*PW + floor(dx)
        dyx = work.tile([128, NM], F32)
        dyx3 = dyx[:].rearrange("p (k i) -> p k i", k=K2, i=HI)
        nc.vector.tensor_scalar(dyx3, kv(flo)[:, :, 0, :], float(PW), None, ALU.mult)
        nc.vector.tensor_tensor(dyx3, dyx3, kv(flo)[:, :, 1, :], ALU.add)

        # ---- Phase 1b: repack dyx [j, m] -> dyx2 [u, (t, jw, m_local)] --
        # stage 1: dyxT[m_local, t, j] via 5 PE transposes of [128, <=128]
        dyxT = work.tile([128, 5 * 128], F32)
        nc.vector.memset(dyxT[:], 0.0)
        for t in range(5):
            wdt = 128 if t < 4 else 64
            ps1 = ps1pool.tile([128, 128], F32)
            nc.tensor.transpose(
                ps1[0:wdt, :], dyx[:, t * 128:t * 128 + wdt], identf[:])
            nc.scalar.copy(dyxT[0:wdt, t * 128:(t + 1) * 128], ps1[0:wdt, :])
        # stage 2: dyx2[u, (t, jw, local)] via 40 transposes of [128, 16]
        dyx2 = work.tile([128, 5 * 8 * 128], F32)
        for t in range(5):
            ps2 = ps2pool.tile([128, 8 * 128], F32)
            for jw in range(8):
                nc.tensor.transpose(
                    ps2[0:16, jw * 128:(jw + 1) * 128],
                    dyxT[:, t * 128 + 16 * jw:t * 128 + 16 * jw + 16],
                    identf[:])
            nc.scalar.copy(dyx2[0:16, t * 1024:(t + 1) * 1024], ps2[0:16, :])

        # idxs[u, m*8 + jw] = idxb + dyx2  (int16), partitions 0-15
        # iterate (t, local, jw): idxs/idxb at t*1024 + local*8 + jw,
        # dyx2 at t*1024 + jw*128 + local
        idxs = live.tile([128, NFREE], I16)
        nc.vector.tensor_tensor(
            idxs[0:16, 0:4096].rearrange("p (t l j) -> p t l j",
                                         t=4, l=128, j=8),
            idxb[0:16, 0:4096].rearrange("p (t l j) -> p t l j",
                                         t=4, l=128, j=8),
            dyx2[0:16, 0:4096].rearrange("p (t j l) -> p t l j",
                                         t=4, j=8, l=128),
            ALU.add)
        nc.vector.tensor_tensor(
            idxs[0:16, 4096:4608].rearrange("p (l j) -> p l j", l=64, j=8),
            idxb[0:16, 4096:4608].rearrange("p (l j) -> p l j", l=64, j=8),
            dyx2[0:16, 4096:5120].rearrange("p (j l) -> p l j",
                                            j=8, l=128)[:, 0:64, :],
            ALU.add)
        # replicate idxs partitions 0-15 -> groups 1..7 (DGE reads the wrap
        # from every 16-partition group)
        for grp in range(1, 8):
            nc.sync.dma_start(idxs[16 * grp:16 * grp + 16, :], idxs[0:16, :])

        # ---- Phase 1c: corner weights w4[j, (k, i, q)] bf16, mask folded
        wy = kv(frac)[:, :, 0, :]      # [128, k, i]
        wx = kv(frac)[:, :, 1, :]
        omy = work.tile([128, NM], F32)
        omyv = omy[:].rearrange("p (k i) -> p k i", k=K2, i=HI)
        nc.vector.tensor_scalar(omyv, wy, 1.0, -1.0, ALU.subtract, ALU.mult)
        omx = work.tile([128, NM], F32)
        omxv = omx[:].rearrange("p (k i) -> p k i", k=K2, i=HI)
        nc.vector.tensor_scalar(omxv, wx, 1.0, -1.0, ALU.subtract, ALU.mult)
        m3 = maskj[:].rearrange("p (k i) -> p k i", k=K2, i=HI)
        wxm0 = work.tile([128, NM], F32)
        wxm0v = wxm0[:].rearrange("p (k i) -> p k i", k=K2, i=HI)
        nc.vector.tensor_tensor(wxm0v, omxv, m3, ALU.mult)
        wxm1 = work.tile([128, NM], F32)
        wxm1v = wxm1[:].rearrange("p (k i) -> p k i", k=K2, i=HI)
        nc.vector.tensor_tensor(wxm1v, wx, m3, ALU.mult)

        w4 = live.tile([128, NM * 4], BF16)
        w4v = w4[:].rearrange("p (k i q) -> p k i q", k=K2, i=HI, q=4)
        nc.vector.tensor_tensor(w4v[:, :, :, 0], omyv, wxm0v, ALU.mult)
        nc.vector.tensor_tensor(w4v[:, :, :, 1], omyv, wxm1v, ALU.mult)
        nc.vector.tensor_tensor(w4v[:, :, :, 2], wy, wxm0v, ALU.mult)
        nc.vector.tensor_tensor(w4v[:, :, :, 3], wy, wxm1v, ALU.mult)

        # ---- Phase 2: gather / combine / transpose / conv ----------------
        ph1.close()
        gpool = ctx.enter_context(tc.tile_pool(name="g", bufs=2))
        p4pool = ctx.enter_context(tc.tile_pool(name="p4", bufs=2))
        s2pool = ctx.enter_context(tc.tile_pool(name="s2", bufs=2))
        stpool = ctx.enter_context(tc.tile_pool(name="st", bufs=2))
        obpool = ctx.enter_context(tc.tile_pool(name="ob", bufs=2))
        tpps = ctx.enter_context(tc.tile_pool(name="tp", bufs=2, space="PSUM"))
        outps = ctx.enter_context(tc.tile_pool(name="ops", bufs=2, space="PSUM"))

        idxs4 = idxs[:].rearrange("p (k i jw) -> p k i jw", k=K2, i=HI, jw=8)
        w4r = w4[:].rearrange("p (k i q) -> p k i q", k=K2, i=HI, q=4)

        with nc.allow_low_precision("bf16 deformable-conv pipeline"):
            for b in range(NBLK):
                out_ps = outps.tile([128, R * 64], F32)
                for k in range(K2):
                    g = gpool.tile([128, R * 256], BF16)
                    gv = g[:].rearrange("p (s e) -> p s e", s=R, e=256)
                    for sub in range(R // RSUB):
                        nc.gpsimd.dma_gather(
                            gv[:, sub * RSUB:(sub + 1) * RSUB, :], src_ap,
                            idxs4[:, k,
                                  b * R + sub * RSUB:b * R + (sub + 1) * RSUB,
                                  :],
                            NIDX, NIDX, elem_size=256,
                            queue_num=(b * K2 * (R // RSUB) + k * (R // RSUB)
                                       + sub) % 4,
                        )
                    # weighted corners: p4 = g * w (w broadcast over c)
                    p4 = p4pool.tile([128, R * 256], BF16)
                    wsl = w4r[:, k, b * R:(b + 1) * R, :]
                    w_b = bass.AP(
                        wsl.tensor, wsl.offset,
                        [wsl.ap[0], [4, R], [0, C], [1, 4]],
                    )
                    nc.vector.tensor_tensor(
                        p4[:].rearrange("p (i c q) -> p i c q", i=R, c=C, q=4),
                        g[:].rearrange("p (i c q) -> p i c q", i=R, c=C, q=4),
                        w_b, ALU.mult)
                    # y-corner sum (pairwise: packed-pair reads keep DVE 2x);
                    # x-corner sum is folded into the matmul (two accumulating
                    # planes e=0/1 share the same block-diag stationary)
                    s2 = s2pool.tile([128, R * C * 2], BF16)
                    p4q = p4[:].rearrange("p (ic q2 e) -> p ic q2 e",
                                          ic=R * C, q2=2, e=2)
                    nc.vector.tensor_tensor(
                        s2[:].rearrange("p (ic e) -> p ic e", ic=R * C, e=2),
                        p4q[:, :, 0, :], p4q[:, :, 1, :], ALU.add)
                    # transpose row-pairs of each e-plane to [(i2, c), j]
                    tp = tpps.tile([128, 2 * 8 * 128], BF16)
                    s2v = s2[:].rearrange("p (h x c e) -> p h x c e",
                                          h=R // 2, x=2, c=C, e=2)
                    for e in range(2):
                        for h in range(R // 2):
                            nc.tensor.transpose(
                                tp[:, (e * 8 + h) * 128:(e * 8 + h + 1) * 128],
                                s2v[:, h, :, :, e], identb[:])
                    st = stpool.tile([128, 2 * 8 * 128], BF16)
                    nc.scalar.copy(st[:], tp[:])
                    for e in range(2):
                        for half in range(2):
                            nc.tensor.matmul(
                                out_ps[:, half * 512:(half + 1) * 512],
                                wk2[:, k * 128:(k + 1) * 128],
                                st[:, e * 1024 + half * 512:
                                   e * 1024 + (half + 1) * 512],
                                start=(k == 0 and e == 0),
                                stop=(k == K2 - 1 and e == 1))
                ob = obpool.tile([128, R * 64], F32)
                nc.scalar.copy(ob[:], out_ps[:])
                for i2 in range(2):
                    dst = bass.AP(
                        out_d.tensor, out_d.offset + (b * R + i2) * W,
                        [out_d.ap[0], [2 * W, R // 2], [1, W]],
                    )
                    nc.sync.dma_start(
                        dst,
                        ob[i2 * 64:(i2 + 1) * 64, :].rearrange(
                            "p (h j) -> p h j", h=R // 2, j=W))

    if not nc.is_finalized():
        nc.finalize()
    return nc


def _quad_image(xn):
    """xn: [C, H, W] f32 -> quad bf16 [NQ*256], entry (y,x) = 2x2 block,
    value order (c, q) with q = yc*2+xc."""
    xpad = np.zeros((PH + 1, PW + 1, C), dtype=BF)
    xpad[PAD:PAD + H, PAD:PAD + W, :] = xn.transpose(1, 2, 0).astype(BF)
    xq = np.empty((PH, PW, C, 4), dtype=BF)
    xq[:, :, :, 0] = xpad[0:PH, 0:PW]
    xq[:, :, :, 1] = xpad[0:PH, 1:PW + 1]
    xq[:, :, :, 2] = xpad[1:PH + 1, 0:PW]
    xq[:, :, :, 3] = xpad[1:PH + 1, 1:PW + 1]
    return np.ascontiguousarray(xq.reshape(-1))


def _static_prep(weight):
    # weight is [O, C_in, KH, KW]; reshape -> [O, C_in, K2]
    wk = weight.reshape(C, C, K2)
    wk2 = np.zeros((128, K2, 128), np.float32)
    for i2 in range(2):
        # rows (i2*64 + c), cols (i2*64 + o) = W[o, c, k]
        wk2[i2 * 64:(i2 + 1) * 64, :, i2 * 64:(i2 + 1) * 64] = (
            wk.transpose(1, 2, 0))
    return wk2.astype(BF).reshape(128, K2 * 128)


def _prep_core(x, offset, mask, wk2, xq_cache, core):
    n, half = core // 2, core % 2
    i0 = half * HI
    if n not in xq_cache:
        xq_cache[n] = _quad_image(x[n])
    offj = np.ascontiguousarray(
        offset[n, :, i0:i0 + HI, :].transpose(2, 0, 1)).reshape(128, 2 * K2 * HI)
    maskj = np.ascontiguousarray(
        mask[n, :, i0:i0 + HI, :].transpose(2, 0, 1)).reshape(128, K2 * HI)

    u = np.arange(16)
    k = np.arange(K2)
    ki, kj = k // 3, k % 3
    i = np.arange(HI)
    jw = np.arange(8)
    # idxb[u, (k, i, jw)] = (PAD+i0+i+ki-1)*PW + PAD + jw*16 + u + kj - 1
    base = ((PAD + i0 + i[None, :, None] + ki[:, None, None] - 1) * PW
            + PAD + jw[None, None, :] * 16 + kj[:, None, None] - 1)  # [k, i, jw]
    idxb = (base[None] + u[:, None, None, None]).reshape(16, -1)
    assert idxb.min() - CLAMP * PW - CLAMP >= 0
    assert idxb.max() + CLAMP * PW + CLAMP < NQ

    return {
        "xq": xq_cache[n],
        "offj": offj,
        "maskj": maskj,
        "idxb": idxb.astype(np.float32).reshape(-1),
        "wk2": wk2,
        "identf": np.eye(128, dtype=np.float32),
        "identb": np.eye(128, dtype=BF),
    }


def _prep_all(x, offset, mask, weight):
    x = np.asarray(x, np.float32)
    offset = np.asarray(offset, np.float32)
    mask = np.asarray(mask, np.float32)
    weight = np.asarray(weight, np.float32)
    wk2 = _static_prep(weight)
    xq_cache = {}
    return [
        _prep_core(x, offset, mask, wk2, xq_cache, core) for core in range(8)
    ]


def _collect(res):
    out = np.empty((N, C, H, W), np.float32)
    for core in range(8):
        n, half = core // 2, core % 2
        out[n, :, half * HI:(half + 1) * HI, :] = (
            res.results[core]["out"].reshape(C, HI, W))
    return out


def kernel_traced(x, offset, mask, weight, trace=True, trace_kwargs=None):
    """Like kernel() but with NTFF tracing; returns (out, BassKernelResults)."""
    if "nc" not in _CACHED:
        _CACHED["nc"] = build_nc()
    in_maps = _prep_all(x, offset, mask, weight)
    res = run_bass_kernel_spmd(_CACHED["nc"], in_maps, list(range(8)),
                               trace=trace, **(trace_kwargs or {}))
    return _collect(res), res


def kernel(x, offset, mask, weight):
    if "nc" not in _CACHED:
        _CACHED["nc"] = build_nc()
    in_maps = _prep_all(x, offset, mask, weight)
    res = run_bass_kernel_spmd(_CACHED["nc"], in_maps, list(range(8)))
    return _collect(res)


# revision 19
# speedup vs baseline: 1.5467x; 1.5467x over previous
"""DeformConv2d (DCNv2) Trainium2 Bass kernel, v2.

Problem: N=4, C_IN=C_OUT=64, H=W=128, 3x3 taps, stride=1, pad=1, dil=1,
modulated deformable conv (torchvision semantics).

Sharding: 8 cores; core = (image n = core//2, row-half = core%2).
Each core computes out[n, :, i0:i0+64, :] from the full image x[n].

v2 design (vs v1 baseline):
  - bf16 "quad" image in DRAM: entry (y,x) holds the 2x2 pixel block
    (y..y+1, x..x+1) x 64ch as 256 bf16 values ordered (c, q) with
    q = yc*2+xc.  One 512B gather descriptor fetches all 4 bilinear
    corners of one (pixel, tap) sample -> half the descriptors and half
    the HBM bytes of v1.
  - index repack j-major -> 16-partition-wrapped via two PE transpose
    stages (v1 used a DRAM bounce with 256B descriptors: ~460us).
  - corner combine: one 2x-mode DVE multiply (weights broadcast over
    channels via stride-0 AP) + one tensor_reduce over the 4-corner
    inner axis.
  - conv: row-pair transposes ([128j, 128(i2,c)] -> [(i2,c), j]) and
    per-tap 128-contraction matmuls with block-diagonal duplicated
    weights -> 2 output pixels per PE column.
"""
import sys

_TRN_REPO = "/opt/trn_rl_repo"
if _TRN_REPO not in sys.path:
    sys.path.insert(0, _TRN_REPO)

import numpy as np
import ml_dtypes

import concourse.bass as bass
import concourse.bacc as bacc
import concourse.tile as tile
import concourse.mybir as mybir
from concourse.bass_utils import run_bass_kernel_spmd
from contextlib import ExitStack

F32 = mybir.dt.float32
BF16 = mybir.dt.bfloat16
I16 = mybir.dt.int16
ALU = mybir.AluOpType
BF = ml_dtypes.bfloat16

N, C, H, W = 4, 64, 128, 128
K2 = 9
PAD = 16                    # coordinate padding on each side
PH = H + 2 * PAD            # 160
PW = W + 2 * PAD            # 160
NQ = PH * PW                # 25600 quad entries
HI = 64                     # rows per core
R = 16                      # rows per gather block
NBLK = HI // R              # 4
RSUB = 8                    # rows per dma_gather call (HW caps 1024 descs)
NIDX = RSUB * W             # descriptors per dma_gather call
CLAMP = 11.0                # |floor(offset)| clamp (pad-region safe)
MAGIC = 12582912.0          # 1.5 * 2**23 for round-to-nearest-even
DMA_SCRATCH = 49152         # SWDGE descriptor carveout (ring = this/16)

_CACHED = {}


def build_nc():
    nc = bacc.Bacc(trn_type="TRN2", debug=False, num_swdge_queues=4,
                   dynamic_dma_scratch_size=DMA_SCRATCH)

    xq_d = nc.dram_tensor("xq", [NQ * 256], BF16, kind="ExternalInput")
    offj_d = nc.dram_tensor("offj", [128, 2 * K2 * HI], F32, kind="ExternalInput").ap()
    maskj_d = nc.dram_tensor("maskj", [128, K2 * HI], F32, kind="ExternalInput").ap()
    idxb_d = nc.dram_tensor("idxb", [16 * K2 * HI * 8], F32, kind="ExternalInput")
    wk2_d = nc.dram_tensor("wk2", [128, K2 * 128], BF16, kind="ExternalInput").ap()
    identf_d = nc.dram_tensor("identf", [128, 128], F32, kind="ExternalInput").ap()
    identb_d = nc.dram_tensor("identb", [128, 128], BF16, kind="ExternalInput").ap()
    out_d = nc.dram_tensor("out", [64, HI * W], F32, kind="ExternalOutput").ap()

    # gather source: quad entries of the padded image
    src_ap = bass.AP(xq_d, 0, [[256, NQ], [1, 256]])

    NM = K2 * HI            # 576 (k, i) pairs
    NFREE = NM * 8          # 4608 descriptors per 16-partition wrap

    with ExitStack() as ctx:
        tc = ctx.enter_context(tile.TileContext(nc))

        const = ctx.enter_context(tc.tile_pool(name="const", bufs=1))
        live = ctx.enter_context(tc.tile_pool(name="live", bufs=1))
        ph1 = ExitStack()
        work = ph1.enter_context(tc.tile_pool(name="work", bufs=1))
        ps1pool = ph1.enter_context(tc.tile_pool(name="ps1", bufs=2, space="PSUM"))
        ps2pool = ph1.enter_context(tc.tile_pool(name="ps2", bufs=2, space="PSUM"))

        identf = const.tile([128, 128], F32)
        nc.sync.dma_start(identf[:], identf_d)
        identb = const.tile([128, 128], BF16)
        nc.sync.dma_start(identb[:], identb_d)
        wk2 = const.tile([128, K2 * 128], BF16)
        nc.sync.dma_start(wk2[:], wk2_d)

        offj = work.tile([128, 2 * K2 * HI], F32)
        nc.sync.dma_start(offj[:], offj_d)
        maskj = work.tile([128, K2 * HI], F32)
        nc.sync.dma_start(maskj[:], maskj_d)
        idxb = work.tile([128, NFREE], F32)
        nc.sync.dma_start(idxb[0:16, :], bass.AP(idxb_d, 0, [[NFREE, 16], [1, NFREE]]))

        # ---- Phase 1a: floor / frac ------------------------------------
        flo = work.tile([128, 2 * K2 * HI], F32)
        nc.vector.tensor_scalar(flo[:], offj[:], MAGIC, None, ALU.add)
        nc.vector.tensor_scalar(flo[:], flo[:], MAGIC, None, ALU.subtract)
        rup = work.tile([128, 2 * K2 * HI], F32)
        nc.vector.tensor_tensor(rup[:], flo[:], offj[:], ALU.is_gt)
        nc.vector.tensor_tensor(flo[:], flo[:], rup[:], ALU.subtract)
        frac = work.tile([128, 2 * K2 * HI], F32)
        nc.vector.tensor_tensor(frac[:], offj[:], flo[:], ALU.subtract)
        nc.vector.tensor_scalar(flo[:], flo[:], -CLAMP, None, ALU.max)
        nc.vector.tensor_scalar(flo[:], flo[:], CLAMP, None, ALU.min)

        def kv(t):  # [128, (k, two, i)]
            return t[:].rearrange("p (k two i) -> p k two i", k=K2, two=2, i=HI)

        # dyx[j, m=(k,i)] = floor(dy)*PW + floor(dx)
        dyx = work.tile([128, NM], F32)
        dyx3 = dyx[:].rearrange("p (k i) -> p k i", k=K2, i=HI)
        nc.vector.tensor_scalar(dyx3, kv(flo)[:, :, 0, :], float(PW), None, ALU.mult)
        nc.vector.tensor_tensor(dyx3, dyx3, kv(flo)[:, :, 1, :], ALU.add)

        # ---- Phase 1b: repack dyx [j, m] -> dyx2 [u, (t, jw, m_local)] --
        # stage 1: dyxT[m_local, t, j] via 5 PE transposes of [128, <=128]
        dyxT = work.tile([128, 5 * 128], F32)
        nc.vector.memset(dyxT[:], 0.0)
        for t in range(5):
            wdt = 128 if t < 4 else 64
            ps1 = ps1pool.tile([128, 128], F32)
            nc.tensor.transpose(
                ps1[0:wdt, :], dyx[:, t * 128:t * 128 + wdt], identf[:])
            nc.scalar.copy(dyxT[0:wdt, t * 128:(t + 1) * 128], ps1[0:wdt, :])
        # stage 2: dyx2[u, (t, jw, local)] via 40 transposes of [128, 16]
        dyx2 = work.tile([128, 5 * 8 * 128], F32)
        for t in range(5):
            ps2 = ps2pool.tile([128, 8 * 128], F32)
            for jw in range(8):
                nc.tensor.transpose(
                    ps2[0:16, jw * 128:(jw + 1) * 128],
                    dyxT[:, t * 128 + 16 * jw:t * 128 + 16 * jw + 16],
                    identf[:])
            nc.scalar.copy(dyx2[0:16, t * 1024:(t + 1) * 1024], ps2[0:16, :])

        # idxs[u, m*8 + jw] = idxb + dyx2  (int16), partitions 0-15
        # iterate (t, local, jw): idxs/idxb at t*1024 + local*8 + jw,
        # dyx2 at t*1024 + jw*128 + local
        idxs = live.tile([128, NFREE], I16)
        nc.vector.tensor_tensor(
            idxs[0:16, 0:4096].rearrange("p (t l j) -> p t l j",
                                         t=4, l=128, j=8),
            idxb[0:16, 0:4096].rearrange("p (t l j) -> p t l j",
                                         t=4, l=128, j=8),
            dyx2[0:16, 0:4096].rearrange("p (t j l) -> p t l j",
                                         t=4, j=8, l=128),
            ALU.add)
        nc.vector.tensor_tensor(
            idxs[0:16, 4096:4608].rearrange("p (l j) -> p l j", l=64, j=8),
            idxb[0:16, 4096:4608].rearrange("p (l j) -> p l j", l=64, j=8),
            dyx2[0:16, 4096:5120].rearrange("p (j l) -> p l j",
                                            j=8, l=128)[:, 0:64, :],
            ALU.add)
        # replicate idxs partitions 0-15 -> groups 1..7 (DGE reads the wrap
        # from every 16-partition group)
        for grp in range(1, 8):
            nc.sync.dma_start(idxs[16 * grp:16 * grp + 16, :], idxs[0:16, :])

        # ---- Phase 1c: corner weights w4[j, (k, i, q)] bf16, mask folded
        wy = kv(frac)[:, :, 0, :]      # [128, k, i]
        wx = kv(frac)[:, :, 1, :]
        omy = work.tile([128, NM], F32)
        omyv = omy[:].rearrange("p (k i) -> p k i", k=K2, i=HI)
        nc.vector.tensor_scalar(omyv, wy, 1.0, -1.0, ALU.subtract, ALU.mult)
        omx = work.tile([128, NM], F32)
        omxv = omx[:].rearrange("p (k i) -> p k i", k=K2, i=HI)
        nc.vector.tensor_scalar(omxv, wx, 1.0, -1.0, ALU.subtract, ALU.mult)
        m3 = maskj[:].rearrange("p (k i) -> p k i", k=K2, i=HI)
        wxm0 = work.tile([128, NM], F32)
        wxm0v = wxm0[:].rearrange("p (k i) -> p k i", k=K2, i=HI)
        nc.vector.tensor_tensor(wxm0v, omxv, m3, ALU.mult)
        wxm1 = work.tile([128, NM], F32)
        wxm1v = wxm1[:].rearrange("p (k i) -> p k i", k=K2, i=HI)
        nc.vector.tensor_tensor(wxm1v, wx, m3, ALU.mult)

        w4 = live.tile([128, NM * 4], BF16)
        w4v = w4[:].rearrange("p (k i q) -> p k i q", k=K2, i=HI, q=4)
        nc.vector.tensor_tensor(w4v[:, :, :, 0], omyv, wxm0v, ALU.mult)
        nc.vector.tensor_tensor(w4v[:, :, :, 1], omyv, wxm1v, ALU.mult)
        nc.vector.tensor_tensor(w4v[:, :, :, 2], wy, wxm0v, ALU.mult)
        nc.vector.tensor_tensor(w4v[:, :, :, 3], wy, wxm1v, ALU.mult)

        # ---- Phase 2: gather / combine / transpose / conv ----------------
        ph1.close()
        gpool = ctx.enter_context(tc.tile_pool(name="g", bufs=4))
        p4pool = ctx.enter_context(tc.tile_pool(name="p4", bufs=2))
        s2pool = ctx.enter_context(tc.tile_pool(name="s2", bufs=2))
        stpool = ctx.enter_context(tc.tile_pool(name="st", bufs=2))
        obpool = ctx.enter_context(tc.tile_pool(name="ob", bufs=2))
        tpps = ctx.enter_context(tc.tile_pool(name="tp", bufs=2, space="PSUM"))
        outps = ctx.enter_context(tc.tile_pool(name="ops", bufs=2, space="PSUM"))

        idxs4 = idxs[:].rearrange("p (k i jw) -> p k i jw", k=K2, i=HI, jw=8)
        w4r = w4[:].rearrange("p (k i q) -> p k i q", k=K2, i=HI, q=4)

        with nc.allow_low_precision("bf16 deformable-conv pipeline"):
            for b in range(NBLK):
                out_ps = outps.tile([128, R * 64], F32)
                for k in range(K2):
                    g = gpool.tile([128, R * 256], BF16)
                    gv = g[:].rearrange("p (s e) -> p s e", s=R, e=256)
                    for sub in range(R // RSUB):
                        nc.gpsimd.dma_gather(
                            gv[:, sub * RSUB:(sub + 1) * RSUB, :], src_ap,
                            idxs4[:, k,
                                  b * R + sub * RSUB:b * R + (sub + 1) * RSUB,
                                  :],
                            NIDX, NIDX, elem_size=256,
                            queue_num=(b * K2 * (R // RSUB) + k * (R // RSUB)
                                       + sub) % 4,
                        )
                    # weighted corners: p4 = g * w (w broadcast over c)
                    p4 = p4pool.tile([128, R * 256], BF16)
                    wsl = w4r[:, k, b * R:(b + 1) * R, :]
                    w_b = bass.AP(
                        wsl.tensor, wsl.offset,
                        [wsl.ap[0], [4, R], [0, C], [1, 4]],
                    )
                    nc.vector.tensor_tensor(
                        p4[:].rearrange("p (i c q) -> p i c q", i=R, c=C, q=4),
                        g[:].rearrange("p (i c q) -> p i c q", i=R, c=C, q=4),
                        w_b, ALU.mult)
                    # y-corner sum (pairwise: packed-pair reads keep DVE 2x);
                    # x-corner sum is folded into the matmul (two accumulating
                    # planes e=0/1 share the same block-diag stationary)
                    s2 = s2pool.tile([128, R * C * 2], BF16)
                    p4q = p4[:].rearrange("p (ic q2 e) -> p ic q2 e",
                                          ic=R * C, q2=2, e=2)
                    nc.vector.tensor_tensor(
                        s2[:].rearrange("p (ic e) -> p ic e", ic=R * C, e=2),
                        p4q[:, :, 0, :], p4q[:, :, 1, :], ALU.add)
                    # transpose row-pairs of each e-plane to [(i2, c), j]
                    tp = tpps.tile([128, 2 * 8 * 128], BF16)
                    s2v = s2[:].rearrange("p (h x c e) -> p h x c e",
                                          h=R // 2, x=2, c=C, e=2)
                    for e in range(2):
                        for h in range(R // 2):
                            nc.tensor.transpose(
                                tp[:, (e * 8 + h) * 128:(e * 8 + h + 1) * 128],
                                s2v[:, h, :, :, e], identb[:])
                    st = stpool.tile([128, 2 * 8 * 128], BF16)
                    nc.scalar.copy(st[:], tp[:])
                    for e in range(2):
                        for half in range(2):
                            nc.tensor.matmul(
                                out_ps[:, half * 512:(half + 1) * 512],
                                wk2[:, k * 128:(k + 1) * 128],
                                st[:, e * 1024 + half * 512:
                                   e * 1024 + (half + 1) * 512],
                                start=(k == 0 and e == 0),
                                stop=(k == K2 - 1 and e == 1))
                ob = obpool.tile([128, R * 64], F32)
                nc.scalar.copy(ob[:], out_ps[:])
                for i2 in range(2):
                    dst = bass.AP(
                        out_d.tensor, out_d.offset + (b * R + i2) * W,
                        [out_d.ap[0], [2 * W, R // 2], [1, W]],
                    )
                    nc.sync.dma_start(
                        dst,
                        ob[i2 * 64:(i2 + 1) * 64, :].rearrange(
                            "p (h j) -> p h j", h=R // 2, j=W))

    if not nc.is_finalized():
        nc.finalize()
    return nc


def _quad_image(xn):
    """xn: [C, H, W] f32 -> quad bf16 [NQ*256], entry (y,x) = 2x2 block,
    value order (c, q) with q = yc*2+xc."""
    xpad = np.zeros((PH + 1, PW + 1, C), dtype=BF)
    xpad[PAD:PAD + H, PAD:PAD + W, :] = xn.transpose(1, 2, 0).astype(BF)
    xq = np.empty((PH, PW, C, 4), dtype=BF)
    xq[:, :, :, 0] = xpad[0:PH, 0:PW]
    xq[:, :, :, 1] = xpad[0:PH, 1:PW + 1]
    xq[:, :, :, 2] = xpad[1:PH + 1, 0:PW]
    xq[:, :, :, 3] = xpad[1:PH + 1, 1:PW + 1]
    return np.ascontiguousarray(xq.reshape(-1))


def _static_prep(weight):
    # weight is [O, C_in, KH, KW]; reshape -> [O, C_in, K2]
    wk = weight.reshape(C, C, K2)
    wk2 = np.zeros((128, K2, 128), np.float32)
    for i2 in range(2):
        # rows (i2*64 + c), cols (i2*64 + o) = W[o, c, k]
        wk2[i2 * 64:(i2 + 1) * 64, :, i2 * 64:(i2 + 1) * 64] = (
            wk.transpose(1, 2, 0))
    return wk2.astype(BF).reshape(128, K2 * 128)


def _prep_core(x, offset, mask, wk2, xq_cache, core):
    n, half = core // 2, core % 2
    i0 = half * HI
    if n not in xq_cache:
        xq_cache[n] = _quad_image(x[n])
    offj = np.ascontiguousarray(
        offset[n, :, i0:i0 + HI, :].transpose(2, 0, 1)).reshape(128, 2 * K2 * HI)
    maskj = np.ascontiguousarray(
        mask[n, :, i0:i0 + HI, :].transpose(2, 0, 1)).reshape(128, K2 * HI)

    u = np.arange(16)
    k = np.arange(K2)
    ki, kj = k // 3, k % 3
    i = np.arange(HI)
    jw = np.arange(8)
    # idxb[u, (k, i, jw)] = (PAD+i0+i+ki-1)*PW + PAD + jw*16 + u + kj - 1
    base = ((PAD + i0 + i[None, :, None] + ki[:, None, None] - 1) * PW
            + PAD + jw[None, None, :] * 16 + kj[:, None, None] - 1)  # [k, i, jw]
    idxb = (base[None] + u[:, None, None, None]).reshape(16, -1)
    assert idxb.min() - CLAMP * PW - CLAMP >= 0
    assert idxb.max() + CLAMP * PW + CLAMP < NQ

    return {
        "xq": xq_cache[n],
        "offj": offj,
        "maskj": maskj,
        "idxb": idxb.astype(np.float32).reshape(-1),
        "wk2": wk2,
        "identf": np.eye(128, dtype=np.float32),
        "identb": np.eye(128, dtype=BF),
    }


def _prep_all(x, offset, mask, weight):
    x = np.asarray(x, np.float32)
    offset = np.asarray(offset, np.float32)
    mask = np.asarray(mask, np.float32)
    weight = np.asarray(weight, np.float32)
    wk2 = _static_prep(weight)
    xq_cache = {}
    return [
        _prep_core(x, offset, mask, wk2, xq_cache, core) for core in range(8)
    ]


def _collect(res):
    out = np.empty((N, C, H, W), np.float32)
    for core in range(8):
        n, half = core // 2, core % 2
        out[n, :, half * HI:(half + 1) * HI, :] = (
            res.results[core]["out"].reshape(C, HI, W))
    return out


def kernel_traced(x, offset, mask, weight, trace=True, trace_kwargs=None):
    """Like kernel() but with NTFF tracing; returns (out, BassKernelResults)."""
    if "nc" not in _CACHED:
        _CACHED["nc"] = build_nc()
    in_maps = _prep_all(x, offset, mask, weight)
    res = run_bass_kernel_spmd(_CACHED["nc"], in_maps, list(range(8)),
                               trace=trace, **(trace_kwargs or {}))
    return _collect(res), res


def kernel(x, offset, mask, weight):
    if "nc" not in _CACHED:
        _CACHED["nc"] = build_nc()
    in_maps = _prep_all(x, offset, mask, weight)
    res = run_bass_kernel_spmd(_CACHED["nc"], in_maps, list(range(8)))
    return _collect(res)
